# revision 31
# baseline (speedup 1.0000x reference)
"""MCR2 (Maximal Coding Rate Reduction) loss kernel for 8 Trainium2 NeuronCores.

Strategy
--------
The loss is built from (k+1) tiny 64x64 Gram matrices reduced over m=262144
samples: G_total = E^T E and per-class G_j = E_j^T E_j (classes partition the
sample set, so G_total = sum_j G_j), followed by slogdet on 64x64 matrices.

Sharding: data-parallel over the sample axis. On the host we sort samples by
class (a Gram is permutation-invariant), pad each class block with zero rows
(zeros contribute nothing to a Gram) so every device gets an identical even
number of 128-row class-pure chunks, and pre-pack each device shard
partition-major so the device DMA is fully contiguous.

Device compute (raw bass, no Tile): chunks are processed in same-class PAIRS
using PE COLUMN TILING: the 128x128 array is split into two independent
128x64 column tiles (tile_position (0,0) and (0,64)). Chunk A's self-loading
Gram matmul runs on tile 0 (output -> PSUM partitions 0:64), chunk B's on
tile 1 (output -> partitions 64:128). Each matmul streams only 64 moving
columns, and the two tiles stream concurrently on separate XBUSes, so a pair
costs ~64 PE cycles instead of the 128 a full-array [A|B]^T[A|B] pair matmul
costs. Per-class accumulation start/stop flags are tracked per tile half.

Run layout (from perfetto trace analysis of prior versions):
- ~6.6us fixed framework preamble (semaphore init + TENSOR_LOAD), then main.
- Input DMA in few LARGE class-pure groups over THREE HWDGE rings (SP,
  Activation, DVE): descriptor generation (~0.65us per group) pipelines
  three deep, so the 16 DMA engines stream at their aggregate ~360GB/s
  (22.5B/ns each). Group sizes ramp up (PE starts early) then down (the
  final grp_sem wait + 900ns DMA->sem propagation covers few pairs).
- Warm-up matmuls sized to the first-group latency keep the PE HAM activity
  window busy from the first instruction so the 1.2->2.4GHz un-throttle
  (~3.4us of sustained activity) is paid as early as possible.
- PSUM: class j accumulates in ps[0:64, j*64:(j+1)*64] (A halves) and
  ps[64:128, j*64:(j+1)*64] (B halves). Classes 0-7 sit in bank 0, class 8
  in bank 1, class 9 in bank 2 (PSUM banks are 512 f32 cols), warmup
  scratch in bank 3. Flush stages are bank-aligned; when a bank's last
  class completes, DVE copies its top rows and ACT its bottom rows to SBUF
  concurrently (plain contiguous copies), then the bank is DMAd out. Only
  the single-class bank-2 stage sits after the final matmul, and its two
  64-partition output DMAs issue on two rings in parallel.

The 8 partial Gram images are summed on the host, where the 11 slogdets of
64x64 matrices (~3 MFLOP, vs ~2.1 GFLOP of Gram work on device) and the
final scalar combine run in float64.

Inputs are rounded to float8-e4m3 for the device matmuls, quartering DMA
bytes. The systematic Gram perturbation largely cancels between the
discriminative and compressive terms: the loss matches the fp32 reference
to ~1.0e-3 relative (measured).
"""

import numpy as np
import ml_dtypes

NCORES = 8
P = 64  # feature dim
NCLASS = 10
CHUNK = 128
GAM1 = 1.0
GAM2 = 1.0
EPS = 0.01

COMPUTE_DTYPE = "float8e4"  # "bfloat16" | "float8e4"
NWARM = 24
# flush stages aligned to PSUM banks (bank = 512 f32 cols; classes 0-3 in
# bank0, 4-7 in bank1, 8-9 in bank2): each stage's copies may run while the
# PE is still writing LATER banks, so stages 1-2 overlap the compute stream
# and only the small 2-class stage sits after the final matmul
FLUSH = ((0, 4), (4, 8), (8, 10))
# f32-col offset of each class's 64-wide PSUM block (+ warmup scratch);
# classes 0-3 bank0, 4-7 bank1, 8-9 bank2
PSUM_COL = tuple([64 * j for j in range(4)]
                 + [512 + 64 * j for j in range(4)]
                 + [1024, 1088])
PSUM_SCRATCH = 1536
PSUM_W = PSUM_SCRATCH + CHUNK

PROFILE = False  # set True (e.g. from test.py) to capture NTFF timing
LAST_EXEC_NS = None
LAST_RESULTS = None

_NP_DT = {
    "float32": np.float32,
    "bfloat16": ml_dtypes.bfloat16,
    "float8e4": ml_dtypes.float8_e4m3,
}

_prog_cache = {}


def _group_plan(C):
    """Input DMA groups: fine-grained and uniform, zippered over THREE DMA
    rings (SP, ACT, Pool). Three rings are needed for aggregate engine
    throughput (~295B/ns vs ~232 on one ring); uniform small groups keep the
    fair-share completion skew between rings (~one group) small so groups
    become available nearly in consumption order. Small first group so the
    PE starts early; small last group so the final 900ns DMA->semaphore
    propagation covers almost no work."""
    plan = [12]
    left = C - 12 - 8
    while left >= 20:
        plan.append(20)
        left -= 20
    if left > 0:
        left -= left % 2
        plan.append(left)
    plan.append(8)
    assert sum(plan) == C and all(g % 2 == 0 for g in plan), (plan, C)
    return plan


def _build_program(chunks_dev, dt_name):
    """Build + compile the per-core raw-bass program (identical across cores)."""
    import concourse.bacc as bacc
    import concourse.mybir as mybir

    C = int(sum(chunks_dev))
    assert C % 2 == 0 and all(n % 2 == 0 for n in chunks_dev)
    dt = getattr(mybir.dt, dt_name)
    f32 = mybir.dt.float32

    nc = bacc.Bacc("TRN2", target_bir_lowering=False, debug=False,
                   num_devices=NCORES)
    x = nc.dram_tensor("x", [CHUNK, C * P], dt, kind="ExternalInput")
    out_d = nc.dram_tensor("out", [CHUNK, NCLASS * P], f32,
                           kind="ExternalOutput")

    classes = []
    for j, n in enumerate(chunks_dev):
        classes += [j] * int(n)
    pairs_total = [int(n) // 2 for n in chunks_dev]
    pair_seen = [0] * NCLASS
    groups = _group_plan(C)

    from contextlib import ExitStack
    with ExitStack() as stack:
        t = stack.enter_context(nc.sbuf_tensor([CHUNK, C * P], dt))
        # never written: garbage contents are fine, it only warms the PE clock
        warm_t = stack.enter_context(nc.sbuf_tensor([CHUNK, CHUNK], dt))
        ps = stack.enter_context(nc.psum_tensor([CHUNK, PSUM_W], f32))
        r = stack.enter_context(nc.sbuf_tensor([CHUNK, NCLASS * P], f32))
        # one semaphore per input DMA: the 16 per-engine slice completions of
        # different DMAs are not FIFO across groups, so a single counting
        # semaphore would let group gi's matmuls run on slices of LATER groups
        grp_sem = [stack.enter_context(nc.semaphore(f"grp_sem_{gi}"))
                   for gi in range(len(groups))]
        pe_sem = stack.enter_context(nc.semaphore())
        dve_sem = stack.enter_context(nc.semaphore())
        act_sem = stack.enter_context(nc.semaphore())
        out_sem = stack.enter_context(nc.semaphore())
        block = stack.enter_context(nc.Block())

        scratch = ps[:, PSUM_SCRATCH:PSUM_SCRATCH + CHUNK]
        starts = np.concatenate([[0], np.cumsum(groups)])[:-1]

        def stage_cols(j0, j1):
            c0 = PSUM_COL[j0]
            return c0, c0 + (j1 - j0) * P

        def issue_inputs(eng, qi):
            for gi, gn in enumerate(groups):
                if gi % 3 == qi:
                    g0 = int(starts[gi])
                    eng.dma_start(
                        t[:, g0 * P:(g0 + gn) * P],
                        x[:, g0 * P:(g0 + gn) * P],
                    ).then_inc(grp_sem[gi], 16)

        @block.sync
        def _(sync):
            issue_inputs(sync, 0)
            # overlapped out-DMAs for stages 0..n-2: deferred until the LAST
            # input group has transferred — issuing them mid-stream steals
            # DMA-engine time from the input tail, which delays the last
            # grp_sem (the critical path) by up to the transfer time
            sync.wait_ge(grp_sem[len(groups) - 1], 16)
            for fi, (j0, j1) in enumerate(FLUSH[:-1]):
                sync.wait_ge(dve_sem, fi + 1)
                sync.wait_ge(act_sem, fi + 1)
                sync.dma_start(out_d[:, j0 * P:j1 * P],
                               r[:, j0 * P:j1 * P]).then_inc(out_sem, 16)

        @block.gpsimd
        def _(gpsimd):
            # third input ring: Pool software-DGE (qPoolDynamic); at the end
            # it carries the final stage's top-half DMA on its empty ring
            issue_inputs(gpsimd, 2)
            fi = len(FLUSH) - 1
            j0, j1 = FLUSH[fi]
            gpsimd.wait_ge(dve_sem, fi + 1)
            gpsimd.dma_start(out_d[0:P, j0 * P:j1 * P],
                             r[0:P, j0 * P:j1 * P]).then_inc(out_sem, 16)

        @block.scalar
        def _(scalar):
            issue_inputs(scalar, 1)
            # bottom (B-half) rows of each flush stage, concurrent with DVE
            for fi, (j0, j1) in enumerate(FLUSH):
                scalar.wait_ge(pe_sem, fi + 1)
                c0, c1 = stage_cols(j0, j1)
                nc.scalar.copy(r[P:CHUNK, j0 * P:j1 * P],
                               ps[P:CHUNK, c0:c1]).then_inc(act_sem, 1)
            fi = len(FLUSH) - 1
            j0, j1 = FLUSH[fi]
            scalar.wait_ge(act_sem, fi + 1)
            scalar.dma_start(out_d[P:CHUNK, j0 * P:j1 * P],
                             r[P:CHUNK, j0 * P:j1 * P]).then_inc(out_sem, 16)

        @block.vector
        def _(vector):
            # top (A-half) rows of each flush stage
            for fi, (j0, j1) in enumerate(FLUSH):
                vector.wait_ge(pe_sem, fi + 1)
                c0, c1 = stage_cols(j0, j1)
                nc.vector.tensor_copy(r[0:P, j0 * P:j1 * P],
                                      ps[0:P, c0:c1]).then_inc(dve_sem, 1)

        @block.tensor
        def _(tensor):
            for _ in range(NWARM):
                nc.tensor.matmul(scratch, warm_t[:], warm_t[:],
                                 start=True, stop=True)
            g0 = 0
            mm = None
            flush_j = {st[1] - 1 for st in FLUSH[:-1]}
            for gi, gn in enumerate(groups):
                tensor.wait_ge(grp_sem[gi], 16)
                for c in range(g0, g0 + gn, 2):
                    j = classes[c]
                    pair_seen[j] += 1
                    first = pair_seen[j] == 1
                    last = pair_seen[j] == pairs_total[j]
                    col = PSUM_COL[j]
                    a = t[:, c * P:(c + 1) * P]
                    b = t[:, (c + 1) * P:(c + 2) * P]
                    nc.tensor.matmul(
                        ps[0:P, col:col + P], a, a,
                        start=first, stop=last, tile_position=(0, 0),
                    )
                    mm = nc.tensor.matmul(
                        ps[P:CHUNK, col:col + P], b, b,
                        start=first, stop=last, tile_position=(0, P),
                    )
                    if last and j in flush_j:
                        # a PSUM bank's last class is complete: release it
                        mm.then_inc(pe_sem, 1)
                g0 += gn
            mm.then_inc(pe_sem, 1)

    nc.compile()
    return nc, {"C": C}


def _pack_shards(embed, targets):
    """Sort by class, split per class across cores, zero-pad to an even
    number of class-pure 128-row chunks per core, pack partition-major."""
    m = embed.shape[0]
    t = np.asarray(targets).astype(np.int64).ravel()
    counts = np.bincount(t, minlength=NCLASS).astype(np.int64)
    order = np.argsort(t, kind="stable")
    se = np.ascontiguousarray(np.asarray(embed, dtype=np.float32)[order])

    # even chunk count per class per device
    chunks_dev = 2 * np.maximum(1, -(-counts // (NCORES * 2 * CHUNK))).astype(int)
    C = int(chunks_dev.sum())
    X = np.zeros((NCORES, C * CHUNK, P), dtype=np.float32)
    cls_ofs = np.concatenate([[0], np.cumsum(counts)])
    row0 = np.concatenate([[0], np.cumsum(chunks_dev * CHUNK)])
    for j in range(NCLASS):
        cj = int(counts[j])
        base, rem = divmod(cj, NCORES)
        sizes = base + (np.arange(NCORES) < rem)
        starts = cls_ofs[j] + np.concatenate([[0], np.cumsum(sizes)[:-1]])
        assert sizes.max() <= chunks_dev[j] * CHUNK
        for d in range(NCORES):
            X[d, row0[j]:row0[j] + sizes[d]] = se[starts[d]:starts[d] + sizes[d]]

    Xc = X.astype(_NP_DT[COMPUTE_DTYPE])
    packed = np.ascontiguousarray(
        Xc.reshape(NCORES, C, CHUNK, P).transpose(0, 2, 1, 3)
        .reshape(NCORES, CHUNK, C * P))
    return packed, counts, tuple(int(v) for v in chunks_dev), m


def _ensure_ntff_hook():
    """The agent image's antenv lacks axon_hooks; synthesize it and register
    the ctypes NTFF profile hook so run_bass_kernel_spmd(trace=True) works."""
    import sys, types
    try:
        import antenv.axon_hooks  # noqa: F401
        return True
    except ImportError:
        pass
    try:
        import antenv
        from trn_agent_boot.trn_boot import _ntff_profile_via_ctypes
        mod = types.ModuleType("antenv.axon_hooks")
        _hook = [None]
        mod.set_axon_ntff_profile_hook = lambda h: _hook.__setitem__(0, h)
        mod.get_axon_ntff_profile_hook = lambda: _hook[0]
        sys.modules["antenv.axon_hooks"] = mod
        antenv.axon_hooks = mod
        inner = _ntff_profile_via_ctypes("/opt/axon/libaxon_pjrt.so")

        def hook(output_dir, device_ids):
            # the .so's profile entry points return -1 until the PJRT backend
            # has run at least one execute in this process — force one
            import jax, jax.numpy as jnp
            jnp.zeros((1,)).block_until_ready()
            return inner(output_dir, device_ids)

        mod.set_axon_ntff_profile_hook(hook)
        return True
    except Exception:
        return False


def kernel(embed, targets):
    global LAST_EXEC_NS, LAST_RESULTS
    packed, counts, chunks_dev, m = _pack_shards(embed, targets)

    key = (chunks_dev, COMPUTE_DTYPE, NWARM)
    if key not in _prog_cache:
        _prog_cache[key] = _build_program(chunks_dev, COMPUTE_DTYPE)
    nc, meta = _prog_cache[key]

    from concourse.bass_utils import run_bass_kernel_spmd
    in_maps = [{"x": packed[d]} for d in range(NCORES)]
    do_trace = bool(PROFILE) and _ensure_ntff_hook()
    res = run_bass_kernel_spmd(nc, in_maps, core_ids=list(range(NCORES)),
                               trace=do_trace)
    LAST_EXEC_NS = res.exec_time_ns
    LAST_RESULTS = res

    # host reduction: per-class Gram = sum over cores of the two 64x64 blocks
    grams = np.zeros((NCLASS, P, P), dtype=np.float64)
    for r in res.results:
        o = np.asarray(r["out"], dtype=np.float64)
        for j in range(NCLASS):
            grams[j] += o[:P, j * P:(j + 1) * P] + o[P:, j * P:(j + 1) * P]

    eye = np.eye(P, dtype=np.float64)
    g_tot = grams.sum(axis=0)
    ld_tot = np.linalg.slogdet(eye + GAM1 * (P / (m * EPS)) * g_tot)[1]
    tr_pi = counts.astype(np.float64) + 1e-8
    compress = 0.0
    for j in range(NCLASS):
        ldj = np.linalg.slogdet(eye + (P / (tr_pi[j] * EPS)) * grams[j])[1]
        compress += ldj * tr_pi[j] / m / 2.0
    loss = GAM2 * (-ld_tot / 2.0) + compress
    return np.array(loss, dtype=np.float32)


# revision 32
# speedup vs baseline: 1.0328x; 1.0328x over previous
"""MCR2 (Maximal Coding Rate Reduction) loss kernel for 8 Trainium2 NeuronCores.

Strategy
--------
The loss is built from (k+1) tiny 64x64 Gram matrices reduced over m=262144
samples: G_total = E^T E and per-class G_j = E_j^T E_j (classes partition the
sample set, so G_total = sum_j G_j), followed by slogdet on 64x64 matrices.

Sharding: data-parallel over the sample axis. On the host we sort samples by
class (a Gram is permutation-invariant), pad each class block with zero rows
(zeros contribute nothing to a Gram) so every device gets an identical even
number of 128-row class-pure chunks, and pre-pack each device shard
partition-major so the device DMA is fully contiguous.

Device compute (raw bass, no Tile): chunks are processed in same-class PAIRS
using PE COLUMN TILING: the 128x128 array is split into two independent
128x64 column tiles (tile_position (0,0) and (0,64)). Chunk A's self-loading
Gram matmul runs on tile 0 (output -> PSUM partitions 0:64), chunk B's on
tile 1 (output -> partitions 64:128). Each matmul streams only 64 moving
columns, and the two tiles stream concurrently on separate XBUSes, so a pair
costs ~64 PE cycles instead of the 128 a full-array [A|B]^T[A|B] pair matmul
costs. Per-class accumulation start/stop flags are tracked per tile half.

Run layout (from perfetto trace analysis of prior versions):
- ~6.6us fixed framework preamble (semaphore init + TENSOR_LOAD), then main.
- Input DMA in few LARGE class-pure groups over THREE HWDGE rings (SP,
  Activation, DVE): descriptor generation (~0.65us per group) pipelines
  three deep, so the 16 DMA engines stream at their aggregate ~360GB/s
  (22.5B/ns each). Group sizes ramp up (PE starts early) then down (the
  final grp_sem wait + 900ns DMA->sem propagation covers few pairs).
- Warm-up matmuls sized to the first-group latency keep the PE HAM activity
  window busy from the first instruction so the 1.2->2.4GHz un-throttle
  (~3.4us of sustained activity) is paid as early as possible.
- PSUM: class j accumulates in ps[0:64, j*64:(j+1)*64] (A halves) and
  ps[64:128, j*64:(j+1)*64] (B halves). Classes 0-7 sit in bank 0, class 8
  in bank 1, class 9 in bank 2 (PSUM banks are 512 f32 cols), warmup
  scratch in bank 3. Flush stages are bank-aligned; when a bank's last
  class completes, DVE copies its top rows and ACT its bottom rows to SBUF
  concurrently (plain contiguous copies), then the bank is DMAd out. Only
  the single-class bank-2 stage sits after the final matmul, and its two
  64-partition output DMAs issue on two rings in parallel.

The 8 partial Gram images are summed on the host, where the 11 slogdets of
64x64 matrices (~3 MFLOP, vs ~2.1 GFLOP of Gram work on device) and the
final scalar combine run in float64.

Inputs are rounded to float8-e4m3 for the device matmuls, quartering DMA
bytes. The systematic Gram perturbation largely cancels between the
discriminative and compressive terms: the loss matches the fp32 reference
to ~1.0e-3 relative (measured).
"""

import numpy as np
import ml_dtypes

NCORES = 8
P = 64  # feature dim
NCLASS = 10
CHUNK = 128
GAM1 = 1.0
GAM2 = 1.0
EPS = 0.01

COMPUTE_DTYPE = "float8e4"  # "bfloat16" | "float8e4"
NWARM = 24
# flush stages aligned to PSUM banks (bank = 512 f32 cols; classes 0-3 in
# bank0, 4-7 in bank1, 8-9 in bank2): each stage's copies may run while the
# PE is still writing LATER banks, so stages 1-2 overlap the compute stream
# and only the small 2-class stage sits after the final matmul
FLUSH = ((0, 4), (4, 8), (8, 9), (9, 10))
# f32-col offset of each class's 64-wide PSUM block (+ warmup scratch);
# classes 0-3 bank0, 4-7 bank1, 8 bank2, 9 bank3 so the single-class final
# stage flushes while nothing else is pending
PSUM_COL = tuple([64 * j for j in range(4)]
                 + [512 + 64 * j for j in range(4)]
                 + [1024, 1536])
PSUM_SCRATCH = 2048
PSUM_W = PSUM_SCRATCH + CHUNK

PROFILE = False  # set True (e.g. from test.py) to capture NTFF timing
LAST_EXEC_NS = None
LAST_RESULTS = None

_NP_DT = {
    "float32": np.float32,
    "bfloat16": ml_dtypes.bfloat16,
    "float8e4": ml_dtypes.float8_e4m3,
}

_prog_cache = {}


def _group_plan(C):
    """Input DMA groups: fine-grained and uniform, zippered over THREE DMA
    rings (SP, ACT, Pool). Three rings are needed for aggregate engine
    throughput (~295B/ns vs ~232 on one ring); uniform small groups keep the
    fair-share completion skew between rings (~one group) small so groups
    become available nearly in consumption order. Small first group so the
    PE starts early; small last group so the final 900ns DMA->semaphore
    propagation covers almost no work."""
    plan = [12]
    left = C - 12 - 8
    while left >= 20:
        plan.append(20)
        left -= 20
    if left > 0:
        left -= left % 2
        plan.append(left)
    plan.append(8)
    assert sum(plan) == C and all(g % 2 == 0 for g in plan), (plan, C)
    return plan


def _build_program(chunks_dev, dt_name):
    """Build + compile the per-core raw-bass program (identical across cores)."""
    import concourse.bacc as bacc
    import concourse.mybir as mybir

    C = int(sum(chunks_dev))
    assert C % 2 == 0 and all(n % 2 == 0 for n in chunks_dev)
    dt = getattr(mybir.dt, dt_name)
    f32 = mybir.dt.float32

    nc = bacc.Bacc("TRN2", target_bir_lowering=False, debug=False,
                   num_devices=NCORES)
    x = nc.dram_tensor("x", [CHUNK, C * P], dt, kind="ExternalInput")
    out_d = nc.dram_tensor("out", [CHUNK, NCLASS * P], f32,
                           kind="ExternalOutput")

    classes = []
    for j, n in enumerate(chunks_dev):
        classes += [j] * int(n)
    pairs_total = [int(n) // 2 for n in chunks_dev]
    pair_seen = [0] * NCLASS
    groups = _group_plan(C)

    from contextlib import ExitStack
    with ExitStack() as stack:
        t = stack.enter_context(nc.sbuf_tensor([CHUNK, C * P], dt))
        # never written: garbage contents are fine, it only warms the PE clock
        warm_t = stack.enter_context(nc.sbuf_tensor([CHUNK, CHUNK], dt))
        ps = stack.enter_context(nc.psum_tensor([CHUNK, PSUM_W], f32))
        r = stack.enter_context(nc.sbuf_tensor([CHUNK, NCLASS * P], f32))
        # one semaphore per input DMA: the 16 per-engine slice completions of
        # different DMAs are not FIFO across groups, so a single counting
        # semaphore would let group gi's matmuls run on slices of LATER groups
        grp_sem = [stack.enter_context(nc.semaphore(f"grp_sem_{gi}"))
                   for gi in range(len(groups))]
        pe_sem = stack.enter_context(nc.semaphore())
        dve_sem = stack.enter_context(nc.semaphore())
        act_sem = stack.enter_context(nc.semaphore())
        out_sem = stack.enter_context(nc.semaphore())
        block = stack.enter_context(nc.Block())

        scratch = ps[:, PSUM_SCRATCH:PSUM_SCRATCH + CHUNK]
        starts = np.concatenate([[0], np.cumsum(groups)])[:-1]

        def stage_cols(j0, j1):
            c0 = PSUM_COL[j0]
            return c0, c0 + (j1 - j0) * P

        def issue_inputs(eng, qi):
            for gi, gn in enumerate(groups):
                if gi % 3 == qi:
                    g0 = int(starts[gi])
                    eng.dma_start(
                        t[:, g0 * P:(g0 + gn) * P],
                        x[:, g0 * P:(g0 + gn) * P],
                    ).then_inc(grp_sem[gi], 16)

        @block.sync
        def _(sync):
            issue_inputs(sync, 0)
            # overlapped out-DMAs for stages 0..n-2: deferred until the LAST
            # input group has transferred — issuing them mid-stream steals
            # DMA-engine time from the input tail, which delays the last
            # grp_sem (the critical path) by up to the transfer time
            sync.wait_ge(grp_sem[len(groups) - 1], 16)
            for fi, (j0, j1) in enumerate(FLUSH[:-1]):
                sync.wait_ge(dve_sem, fi + 1)
                sync.wait_ge(act_sem, fi + 1)
                sync.dma_start(out_d[:, j0 * P:j1 * P],
                               r[:, j0 * P:j1 * P]).then_inc(out_sem, 16)

        @block.gpsimd
        def _(gpsimd):
            # third input ring: Pool software-DGE (qPoolDynamic); at the end
            # it carries the final stage's top-half DMA on its empty ring
            issue_inputs(gpsimd, 2)
            fi = len(FLUSH) - 1
            j0, j1 = FLUSH[fi]
            gpsimd.wait_ge(dve_sem, fi + 1)
            gpsimd.dma_start(out_d[0:P, j0 * P:j1 * P],
                             r[0:P, j0 * P:j1 * P]).then_inc(out_sem, 16)

        @block.scalar
        def _(scalar):
            issue_inputs(scalar, 1)
            # bottom (B-half) rows of each flush stage, concurrent with DVE
            for fi, (j0, j1) in enumerate(FLUSH):
                scalar.wait_ge(pe_sem, fi + 1)
                c0, c1 = stage_cols(j0, j1)
                nc.scalar.copy(r[P:CHUNK, j0 * P:j1 * P],
                               ps[P:CHUNK, c0:c1]).then_inc(act_sem, 1)
            fi = len(FLUSH) - 1
            j0, j1 = FLUSH[fi]
            scalar.wait_ge(act_sem, fi + 1)
            scalar.dma_start(out_d[P:CHUNK, j0 * P:j1 * P],
                             r[P:CHUNK, j0 * P:j1 * P]).then_inc(out_sem, 16)

        @block.vector
        def _(vector):
            # top (A-half) rows of each flush stage
            for fi, (j0, j1) in enumerate(FLUSH):
                vector.wait_ge(pe_sem, fi + 1)
                c0, c1 = stage_cols(j0, j1)
                nc.vector.tensor_copy(r[0:P, j0 * P:j1 * P],
                                      ps[0:P, c0:c1]).then_inc(dve_sem, 1)

        @block.tensor
        def _(tensor):
            for _ in range(NWARM):
                nc.tensor.matmul(scratch, warm_t[:], warm_t[:],
                                 start=True, stop=True)
            g0 = 0
            mm = None
            flush_j = {st[1] - 1 for st in FLUSH[:-1]}
            for gi, gn in enumerate(groups):
                tensor.wait_ge(grp_sem[gi], 16)
                for c in range(g0, g0 + gn, 2):
                    j = classes[c]
                    pair_seen[j] += 1
                    first = pair_seen[j] == 1
                    last = pair_seen[j] == pairs_total[j]
                    col = PSUM_COL[j]
                    a = t[:, c * P:(c + 1) * P]
                    b = t[:, (c + 1) * P:(c + 2) * P]
                    nc.tensor.matmul(
                        ps[0:P, col:col + P], a, a,
                        start=first, stop=last, tile_position=(0, 0),
                    )
                    mm = nc.tensor.matmul(
                        ps[P:CHUNK, col:col + P], b, b,
                        start=first, stop=last, tile_position=(0, P),
                    )
                    if last and j in flush_j:
                        # a PSUM bank's last class is complete: release it
                        mm.then_inc(pe_sem, 1)
                g0 += gn
            mm.then_inc(pe_sem, 1)

    nc.compile()
    return nc, {"C": C}


def _pack_shards(embed, targets):
    """Sort by class, split per class across cores, zero-pad to an even
    number of class-pure 128-row chunks per core, pack partition-major."""
    m = embed.shape[0]
    t = np.asarray(targets).astype(np.int64).ravel()
    counts = np.bincount(t, minlength=NCLASS).astype(np.int64)
    order = np.argsort(t, kind="stable")
    se = np.ascontiguousarray(np.asarray(embed, dtype=np.float32)[order])

    # even chunk count per class per device
    chunks_dev = 2 * np.maximum(1, -(-counts // (NCORES * 2 * CHUNK))).astype(int)
    C = int(chunks_dev.sum())
    X = np.zeros((NCORES, C * CHUNK, P), dtype=np.float32)
    cls_ofs = np.concatenate([[0], np.cumsum(counts)])
    row0 = np.concatenate([[0], np.cumsum(chunks_dev * CHUNK)])
    for j in range(NCLASS):
        cj = int(counts[j])
        base, rem = divmod(cj, NCORES)
        sizes = base + (np.arange(NCORES) < rem)
        starts = cls_ofs[j] + np.concatenate([[0], np.cumsum(sizes)[:-1]])
        assert sizes.max() <= chunks_dev[j] * CHUNK
        for d in range(NCORES):
            X[d, row0[j]:row0[j] + sizes[d]] = se[starts[d]:starts[d] + sizes[d]]

    Xc = X.astype(_NP_DT[COMPUTE_DTYPE])
    packed = np.ascontiguousarray(
        Xc.reshape(NCORES, C, CHUNK, P).transpose(0, 2, 1, 3)
        .reshape(NCORES, CHUNK, C * P))
    return packed, counts, tuple(int(v) for v in chunks_dev), m


def _ensure_ntff_hook():
    """The agent image's antenv lacks axon_hooks; synthesize it and register
    the ctypes NTFF profile hook so run_bass_kernel_spmd(trace=True) works."""
    import sys, types
    try:
        import antenv.axon_hooks  # noqa: F401
        return True
    except ImportError:
        pass
    try:
        import antenv
        from trn_agent_boot.trn_boot import _ntff_profile_via_ctypes
        mod = types.ModuleType("antenv.axon_hooks")
        _hook = [None]
        mod.set_axon_ntff_profile_hook = lambda h: _hook.__setitem__(0, h)
        mod.get_axon_ntff_profile_hook = lambda: _hook[0]
        sys.modules["antenv.axon_hooks"] = mod
        antenv.axon_hooks = mod
        inner = _ntff_profile_via_ctypes("/opt/axon/libaxon_pjrt.so")

        def hook(output_dir, device_ids):
            # the .so's profile entry points return -1 until the PJRT backend
            # has run at least one execute in this process — force one
            import jax, jax.numpy as jnp
            jnp.zeros((1,)).block_until_ready()
            return inner(output_dir, device_ids)

        mod.set_axon_ntff_profile_hook(hook)
        return True
    except Exception:
        return False


def kernel(embed, targets):
    global LAST_EXEC_NS, LAST_RESULTS
    packed, counts, chunks_dev, m = _pack_shards(embed, targets)

    key = (chunks_dev, COMPUTE_DTYPE, NWARM)
    if key not in _prog_cache:
        _prog_cache[key] = _build_program(chunks_dev, COMPUTE_DTYPE)
    nc, meta = _prog_cache[key]

    from concourse.bass_utils import run_bass_kernel_spmd
    in_maps = [{"x": packed[d]} for d in range(NCORES)]
    do_trace = bool(PROFILE) and _ensure_ntff_hook()
    res = run_bass_kernel_spmd(nc, in_maps, core_ids=list(range(NCORES)),
                               trace=do_trace)
    LAST_EXEC_NS = res.exec_time_ns
    LAST_RESULTS = res

    # host reduction: per-class Gram = sum over cores of the two 64x64 blocks
    grams = np.zeros((NCLASS, P, P), dtype=np.float64)
    for r in res.results:
        o = np.asarray(r["out"], dtype=np.float64)
        for j in range(NCLASS):
            grams[j] += o[:P, j * P:(j + 1) * P] + o[P:, j * P:(j + 1) * P]

    eye = np.eye(P, dtype=np.float64)
    g_tot = grams.sum(axis=0)
    ld_tot = np.linalg.slogdet(eye + GAM1 * (P / (m * EPS)) * g_tot)[1]
    tr_pi = counts.astype(np.float64) + 1e-8
    compress = 0.0
    for j in range(NCLASS):
        ldj = np.linalg.slogdet(eye + (P / (tr_pi[j] * EPS)) * grams[j])[1]
        compress += ldj * tr_pi[j] / m / 2.0
    loss = GAM2 * (-ld_tot / 2.0) + compress
    return np.array(loss, dtype=np.float32)
